# revision 21
# baseline (speedup 1.0000x reference)
"""Trainium2 Bass kernel for the cross-attention block nn_CA_54889682043704.

Reference computation (B=4, C=512, N=M=4096, da=128):
    q = w_qk @ x; k = w_qk @ y; v = w_v @ y + b_v
    attn = softmax((q^T k)/sqrt(da), axis=M)
    out = relu(BN(w_t @ (v @ attn^T) + b_t))  ->  [B, N, C]

Sharding: (batch b, query-half h) -> 8 cores, collective-free; each core
does the full attention for 2048 queries of one batch element.

Key reassociation: v @ attn^T = W_v (Y P), so G = Y.P is computed on
device and W_v folds into the output projection host-side
(W2 = (BN.w_t).W_v). The v-projection and its PSUM evacuations vanish;
Y^T is DMA'd in a second (m-major) fp8 layout as the G stationary.

fp8 (e4m3) DoubleRow everywhere except the da=128-contraction energy:
    k/q : dual-fp8 over chunk-pairs of C -> bf16 SBUF
    E^T [m128, 2, n512] = k-slice^T q-slice  (bf16, fp32 PSUM pair tiles)
    P   = exp(E^T * scale/1024) -> fp8       (ACT; max-subtraction skipped:
          energies are O(1), exp in [0.1, 11] sits mid-e4m3)
    G  += ytm-pair^T P-pair                  (4 c'-chunks x 16 key-pairs)
    g8  = G * 2^-6 -> fp8
    T   = g8-pair^T W2-pair                  (fp32 PSUM)
    dnT[n,4] = pt-slices^T (0.5*ones)        (transposed denominator burst)
    out = relu(T * recip[n] + bias)  -> bf16 (DVE affine + relu)
Weights are pre-scaled by 2^5 before e4m3 quantization (raw ~0.02 values
would land subnormal); all scales are powers of two folded into the exp
scale, the g8 copy and the 0.5 ones weight, so they cancel exactly.

Schedule (per core, one continuous exp stream ~72us):
  - PSUM banks (8): psA 1x[128,2,512] (even-pair energies, 1024-wide exp)
    + psB 2x[128,512] (odd-pair energies 512-wide, k/q proj, T tiles)
    + psG 4x[128,512] (G accumulators; the dnT burst borrows a freed buf
    at each tile boundary while the next tile's G stream lags behind).
  - The G stream runs LAG=3 pairs behind the energy/exp stream (deferred
    emission): the PE is in-order, so the boundary chain (G stop -> g8
    copies -> dnT burst -> recip) never blocks the next tile's energies.
  - All pt pair tiles of a tile stay resident (20-buf pool) so the
    denominator burst can run at the boundary from SBUF.
  - DMA: ~14 consolidated transfers ordered just-in-time (transfers
    serialize at ~17us); k/q projections are emitted inside the pair loop
    so a late transfer never heads-of-line-blocks the exp stream.
  - Tails (T matmul + epilogue) interleave at pairs {5,8,11,14} of the
    next tile; the last tile drains on ACT+DVE in parallel.
"""

import sys

for _p in ("/opt/trn_rl_repo", "/root/.axon_site/_ro/trn_rl_repo"):
    if _p not in sys.path:
        sys.path.append(_p)

import math
import numpy as np
import ml_dtypes

import concourse.bacc as bacc
import concourse.bass as bass
import concourse.mybir as mybir
from concourse import tile
from concourse.bass_utils import run_bass_kernel_spmd

B, C, N, M = 4, 512, 4096, 4096
DA = 128
NCORES = 8
NL = N // 2            # queries per core
NTILES = NL // 512     # 4 query tiles per core
MP = M // 256          # 16 key pairs
BN_EPS = 1e-5
ESCALE = 1.0 / (math.sqrt(DA) * 1024.0)   # folds the 2^5 q/k weight scales
WS = 32.0                                 # 2^5 weight pre-scale
CH = 0.5                                  # T_psum = (denom/2) * out-tilde

BF16 = mybir.dt.bfloat16
F32 = mybir.dt.float32
FP8 = mybir.dt.float8e4
NP_BF16 = ml_dtypes.bfloat16
NP_FP8 = ml_dtypes.float8_e4m3
PSUM = bass.MemorySpace.PSUM
DR = mybir.MatmulPerfMode.DoubleRow
EXP = mybir.ActivationFunctionType.Exp


def build_program():
    nc = bacc.Bacc("TRN2", target_bir_lowering=False, debug=False,
                   num_devices=NCORES)

    # big-tile layouts: dim1 is always the DoubleRow contraction pair
    xc_d = nc.dram_tensor("xc", [128, 2, 2, NL], FP8, kind="ExternalInput").ap()
    yc_d = nc.dram_tensor("yc", [128, 2, 2, M], FP8, kind="ExternalInput").ap()
    ym_d = nc.dram_tensor("ym", [128, 2, MP, C], FP8,
                          kind="ExternalInput").ap()
    wk_d = nc.dram_tensor("wk8", [128, 2, 2, DA], FP8,
                          kind="ExternalInput").ap()
    w2_d = nc.dram_tensor("w28", [128, 2, 2, C], FP8,
                          kind="ExternalInput").ap()
    bb_d = nc.dram_tensor("biasb", [128, C], F32, kind="ExternalInput").ap()
    out_d = nc.dram_tensor("out", [NL, C], BF16, kind="ExternalOutput").ap()

    with tile.TileContext(nc) as tc:
        with (
            tc.tile_pool(name="persist", bufs=1) as wp,
            tc.tile_pool(name="ptp", bufs=20) as ptp,
            tc.tile_pool(name="ssb", bufs=4) as ssbp,
            tc.tile_pool(name="ep", bufs=2) as ep,
            tc.tile_pool(name="op", bufs=4) as op_,
            tc.tile_pool(name="psA", bufs=1, space=PSUM) as psA,
            tc.tile_pool(name="psB", bufs=2, space=PSUM) as psB,
            tc.tile_pool(name="psG", bufs=4, space=PSUM) as psG,
        ):
            ones8 = wp.tile([128, 2, 1], FP8, tag="ones8", name="ones8")
            nc.vector.memset(ones8[:], CH)  # folds the 1/2 T_psum scale
            zb = wp.tile([128, 1], F32, tag="zb", name="zb")
            nc.vector.memset(zb[:], 0.0)

            # --- DMA: the engines serialize all transfers (~17us of input),
            # so the stream is ordered just-in-time for the first m-loop:
            # y/ym blocks interleaved at consumption pace, x-rest and the
            # tail weights (not needed until tile 1 / the first tail) last.
            wkb = wp.tile([128, 2, 2, DA], FP8, tag="wkb", name="wkb")
            nc.sync.dma_start(out=wkb[:], in_=wk_d[:])
            wkt = [wkb[:, :, p, :] for p in range(2)]
            ytb = wp.tile([128, 2, 2, M], FP8, tag="ytb", name="ytb")
            xtb = wp.tile([128, 2, 2, NL], FP8, tag="xtb", name="xtb")
            ymb = wp.tile([128, 2, MP, C], FP8, tag="ymb", name="ymb")
            nc.gpsimd.dma_start(out=ytb[:, :, :, 0:512],
                              in_=yc_d[:, :, :, 0:512])
            nc.sync.dma_start(out=xtb[:, :, :, 0:512],
                              in_=xc_d[:, :, :, 0:512])
            nc.sync.dma_start(out=ytb[:, :, :, 512:1024],
                              in_=yc_d[:, :, :, 512:1024])
            for g4 in range(4):
                nc.sync.dma_start(out=ymb[:, :, g4 * 4:(g4 + 1) * 4, :],
                                  in_=ym_d[:, :, g4 * 4:(g4 + 1) * 4, :])
                if g4 < 3:
                    nc.sync.dma_start(
                        out=ytb[:, :, :, (g4 + 1) * 1024:(g4 + 2) * 1024],
                        in_=yc_d[:, :, :, (g4 + 1) * 1024:(g4 + 2) * 1024])
            nc.sync.dma_start(out=xtb[:, :, :, 512:NL],
                              in_=xc_d[:, :, :, 512:NL])
            w2b = wp.tile([128, 2, 2, C], FP8, tag="w2b", name="w2b")
            nc.sync.dma_start(out=w2b[:], in_=w2_d[:])
            w2t = [w2b[:, :, p, :] for p in range(2)]
            biasb = wp.tile([128, C], F32, tag="biasb", name="biasb")
            nc.sync.dma_start(out=biasb[:], in_=bb_d[:])
            yt = [ytb[:, :, p, :] for p in range(2)]
            xt = [xtb[:, :, p, :] for p in range(2)]
            ymt = [ymb[:, :, jj, :] for jj in range(MP)]

            k_sb = wp.tile([128, M], BF16, tag="ksb", name="ksb")
            q_sb = wp.tile([128, NL], BF16, tag="qsb", name="qsb")

            # --- k / q projections: dual-fp8 over the two chunk-pairs
            def proj(dst, src, blk, name, on_act=False):
                ps = psB.tile([128, 512], F32, tag="eb", name=name)
                for p in range(2):
                    nc.tensor.matmul(ps[:], lhsT=wkt[p],
                                     rhs=src[p][:, :, blk * 512:(blk + 1) * 512],
                                     start=(p == 0), stop=(p == 1),
                                     perf_mode=DR)
                if on_act:
                    nc.scalar.copy(dst[:, blk * 512:(blk + 1) * 512], ps[:])
                else:
                    nc.vector.tensor_copy(dst[:, blk * 512:(blk + 1) * 512],
                                          ps[:])

            # only the first k/q blocks up front; the rest interleave into
            # the pair loops just-in-time (their DMA lands progressively,
            # and the PE executes in order). q0's copy rides ACT so it does
            # not queue behind k0's on the DVE.
            proj(k_sb, yt, 0, "kps0")
            proj(q_sb, xt, 0, "qps0", on_act=True)

            # --- attention m-loops; tails of tile nt interleave into tile
            # nt+1's pair loop so the PE/ACT never drain at boundaries
            def make_tail(nt, recip, s_sb):
                last = nt == NTILES - 1

                def tail_g(g):
                    n0 = nt * 512
                    t_ps = psB.tile([128, C], F32, tag="eb", name=f"t{nt}_{g}")
                    for p in range(2):
                        nc.tensor.matmul(
                            t_ps[:],
                            lhsT=s_sb[p][:, :, g * 128:(g + 1) * 128],
                            rhs=w2t[p],
                            start=(p == 0), stop=(p == 1), perf_mode=DR)
                    o32 = op_.tile([128, C], F32, tag="o32", name=f"o32_{nt}_{g}")
                    nc.vector.affine_then_add(
                        o32[:], t_ps[:], biasb[:],
                        scale=recip[:, g:g + 1], bias=0.0)
                    o = op_.tile([128, C], BF16, tag="o", name=f"o{nt}_{g}")
                    if last:
                        nc.scalar.activation(
                            o[:], o32[:], mybir.ActivationFunctionType.Relu)
                    else:
                        nc.vector.tensor_scalar_max(o[:], o32[:], 0.0)
                    if last:
                        eng = nc.gpsimd if g < 2 else nc.sync
                    else:
                        eng = nc.gpsimd if g % 2 else nc.sync
                    eng.dma_start(
                        out=out_d[n0 + g * 128:n0 + (g + 1) * 128, :],
                        in_=o[:])

                return [lambda g=g: tail_g(g) for g in range(4)]

            # The G/dn matmul stream runs LAG pairs behind the energy/exp
            # stream (deferred emission): the PE executes in order, so the
            # tile-boundary chain (G stop -> dn_sb/s_sb copies -> next G
            # start) never blocks the next tile's energies, and the exp
            # stream stays continuous across tile boundaries.
            LAG = 3
            deferred = []       # (closure emitting G+dn for one pair)
            pending_tails = []

            def boundary(nt, g_ps, pts):
                # G -> fp8 pairs first (they gate the next tile's G stream
                # AND free the psG buf the dn burst lands in)
                last = nt == NTILES - 1
                s_sb = []
                for p in range(2):
                    t = ssbp.tile([128, 2, 512], FP8, tag="ssb",
                                  name=f"ssb{nt}_{p}")
                    for j in range(2):
                        # last tile: 3 of 4 copies ride the (now idle) ACT
                        # so the tail's T matmuls start sooner
                        if last and j == 0:
                            nc.scalar.mul(t[:, j, :], g_ps[2 * p + j][:],
                                          mul=1.0 / 64.0)
                        else:
                            nc.vector.tensor_scalar_mul(
                                t[:, j, :], g_ps[2 * p + j][:], 1.0 / 64.0)
                    s_sb.append(t)
                # transposed denominator burst: all 16 pt pair tiles are
                # still in SBUF; dnT[n,g] accumulates in a freed psG buf
                # (the next tile's G stream is LAG pairs behind)
                dnt = psG.tile([128, 4], F32, tag="s", name=f"dnt{nt}")
                for pj in range(MP):
                    for g in range(4):
                        nc.tensor.matmul(
                            dnt[:, g:g + 1],
                            lhsT=pts[pj][:, :, g * 128:(g + 1) * 128],
                            rhs=ones8[:],
                            start=(pj == 0), stop=(pj == MP - 1),
                            perf_mode=DR)
                recip = ep.tile([128, 4], F32, tag="recip", name=f"recip{nt}")
                nc.vector.reciprocal(recip[:], dnt[:])
                return make_tail(nt, recip, s_sb)

            for nt in range(NTILES):
                n0 = nt * 512
                g_ps = [psG.tile([128, 512], F32, tag="s", name=f"g{nt}_{ci}")
                        for ci in range(4)]
                pts = []
                for pj in range(MP):
                    if nt == 0 and pj % 2 == 0 and 1 + pj // 2 <= 7:
                        proj(k_sb, yt, 1 + pj // 2, f"kps{1 + pj // 2}")
                    if pj == 10 and nt < NTILES - 1:
                        proj(q_sb, xt, nt + 1, f"qps{nt + 1}")
                    pt = ptp.tile([128, 2, 512], FP8, tag="pt",
                                  name=f"pt{nt}_{pj}")
                    pts.append(pt)
                    if pj % 2 == 0:
                        # even pairs: two-bank pair tile, one 1024-wide exp
                        et = psA.tile([128, 2, 512], F32, tag="et",
                                      name=f"et{nt}_{pj}")
                        for j in range(2):
                            mj = 2 * pj + j
                            nc.tensor.matmul(
                                et[:, j, :],
                                lhsT=k_sb[:, mj * 128:(mj + 1) * 128],
                                rhs=q_sb[:, n0:n0 + 512],
                                start=True, stop=True)
                        nc.scalar.activation(pt[:], et[:], EXP,
                                             bias=zb[:], scale=ESCALE)
                    else:
                        # odd pairs: two single-bank tiles, 512-wide exps
                        for j in range(2):
                            mj = 2 * pj + j
                            et = psB.tile([128, 512], F32, tag="eb",
                                          name=f"et{nt}_{mj}")
                            nc.tensor.matmul(
                                et[:],
                                lhsT=k_sb[:, mj * 128:(mj + 1) * 128],
                                rhs=q_sb[:, n0:n0 + 512],
                                start=True, stop=True)
                            nc.scalar.activation(pt[:, j, :], et[:], EXP,
                                                 bias=zb[:], scale=ESCALE)

                    def gwork(nt=nt, pj=pj, pt=pt, g_ps=g_ps, pts=pts):
                        for ci in range(4):
                            nc.tensor.matmul(
                                g_ps[ci][:],
                                lhsT=ymt[pj][:, :, ci * 128:(ci + 1) * 128],
                                rhs=pt[:],
                                start=(pj == 0), stop=(pj == MP - 1),
                                perf_mode=DR)
                        if pj == MP - 1:
                            pending_tails.extend(boundary(nt, g_ps, pts))

                    deferred.append(gwork)
                    # tile 3: drain the lag to 1 before the final pairs so
                    # only one G + the dn burst trail the last exp
                    lag = 1 if (nt == NTILES - 1 and pj >= 12) else LAG
                    while len(deferred) > lag:
                        deferred.pop(0)()
                    if pending_tails and pj in (4, 7, 10, 13):
                        pending_tails.pop(0)()
            for d in deferred:
                d()
            for t in pending_tails:
                t()

    nc.compile()
    return nc


_PROG = None


def _get_prog():
    global _PROG
    if _PROG is None:
        _PROG = build_program()
    return _PROG


def _pairs(a, width):
    """[C, width] -> [128, 2, 2, width] big-tile pair layout (fp8):
    [i, j, p, w] = a[(2p + j) * 128 + i, w]."""
    return np.ascontiguousarray(
        a.reshape(2, 2, 128, width).transpose(2, 1, 0, 3)).astype(NP_FP8)


def _prep_in_maps(x, y, w_qk, w_v, b_v, w_t, b_t, gamma, beta, run_mean,
                  run_var):
    f32 = lambda a: np.asarray(a, dtype=np.float32)
    x, y = f32(x), f32(y)
    w_qk, w_v, b_v = f32(w_qk), f32(w_v), f32(b_v)
    w_t, b_t = f32(w_t), f32(b_t)
    gamma, beta = f32(gamma), f32(beta)
    run_mean, run_var = f32(run_mean), f32(run_var)

    inv = gamma / np.sqrt(run_var + BN_EPS)
    # b_v folds through attention (softmax rows sum to 1); W_v folds into
    # the BN-scaled output projection: W2 = (w_t * inv) @ w_v
    bias_eff = (w_t @ b_v + b_t) * inv + beta - run_mean * inv
    w2 = (w_t * inv[:, None]) @ w_v

    wk8 = _pairs(w_qk.T * WS, DA)
    w28 = _pairs(w2.T * WS, C)
    biasb = np.ascontiguousarray(
        np.broadcast_to(bias_eff.astype(np.float32), (128, C)))

    x8 = [None] * B
    y8 = [None] * B
    ym8 = [None] * B
    for b in range(B):
        x8[b] = _pairs(x[b], N)
        y8[b] = _pairs(y[b], M)
        # m-major pairs: [i, j, mp, c] = y[c, 256 mp + 128 j + i]
        ym8[b] = np.ascontiguousarray(
            y[b].T.reshape(MP, 2, 128, C).transpose(2, 1, 0, 3)).astype(NP_FP8)

    in_maps = []
    for core in range(NCORES):
        b, h = divmod(core, 2)
        in_maps.append({
            "xc": np.ascontiguousarray(x8[b][:, :, :, h * NL:(h + 1) * NL]),
            "yc": y8[b], "ym": ym8[b],
            "wk8": wk8, "w28": w28, "biasb": biasb,
        })
    return in_maps


def run(trace=False, **inputs):
    nc = _get_prog()
    in_maps = _prep_in_maps(**inputs)
    res = run_bass_kernel_spmd(nc, in_maps, core_ids=list(range(NCORES)),
                               trace=trace)
    out = np.empty((B, N, C), np.float32)
    for core in range(NCORES):
        b, h = divmod(core, 2)
        out[b, h * NL:(h + 1) * NL, :] = np.asarray(
            res.results[core]["out"]).astype(np.float32)
    return out, res


def kernel(**inputs):
    out, _ = run(trace=False, **inputs)
    return out
